# revision 4
# baseline (speedup 1.0000x reference)
"""Trainium2 Bass kernel for CrossGeometricStructureEmbedding.

Math: for each point n and anchor k:
  d_idx = |p_n - a_k| / 0.2; a_idx = atan2(|u x v|, u.v) * 180/(15*pi)
  out[n] = max_k(Wd@emb(d_idx)) + max_k(Wa@emb(a_idx)) + bd + ba
The 256-dim sinusoidal embedding + projection is compressed through a
Fourier-cosine basis: emb(x) ~= cosbasis(u(x)) @ C with u an affine map of
the raw index, so no arccos/arcsin chain is needed on device - the geometry
computes only dist and angle. M_D = M_A = 64 basis rows (128 partitions).

Device structure (8 cores, N=4096 points sharded 512/core, 64 chunks of
512 (point, anchor) pairs grouped into 8 blocks):
  - geometry pipelined per point-group (4 groups) so the chunk pipeline
    starts early.
  - basis built per BLOCK ([128, 4096]-free ops, fixed costs amortized 8x):
    ACT affine (per-partition scale/bias) -> ACT i32 round -> Pool in-place
    subtract (frac) -> ACT Sin -> float32r.
  - f32r weight-stationary matmul runs (PE hides LDWEIGHTS under matmuls;
    bf16/fp8 pay it serially).
  - k-max on DVE: 64 tensor_reduces of 2048-free over whole 4-bank views.
"""
import sys

sys.path.insert(0, "/opt/trn_rl_repo")

import numpy as np
import ml_dtypes
import concourse.bacc as bacc
import concourse.bass as bass
import concourse.tile as tile
from concourse import mybir
from concourse.bass_utils import run_bass_kernel_spmd

F32 = mybir.dt.float32
F32R = mybir.dt.float32r
BF16 = mybir.dt.bfloat16
I32 = mybir.dt.int32
AF = mybir.ActivationFunctionType
OP = mybir.AluOpType

NCORES = 8
N = 4096
NC_PTS = N // NCORES          # 512 points per core
K = 64
HIDDEN = 256
SIGMA_D = 0.2
SIGMA_A = 15.0
FACTOR_A = 180.0 / (SIGMA_A * np.pi)
TWO_PI = float(2.0 * np.pi)

M_D, M_A = 64, 64
MB = M_D + M_A                # 128 basis rows
LO_D, HI_D = -2.0, 42.0       # x_d = 5*dist = d_idx in [0.16, 36.4]
LO_A, HI_A = -1.0, 13.0       # x_a = FACTOR_A*angle in [0, 12]

_DIV = np.exp(np.arange(0, HIDDEN, 2) * (-np.log(10000.0) / HIDDEN))  # (128,)


def _fit_fourier(lo, hi, m, tlo, thi, grid_n=12000):
    xg = np.linspace(lo, hi, grid_n)
    u = (xg - lo) / (hi - lo)
    B = np.cos(np.outer(u, np.arange(m)) * np.pi)
    om = xg[:, None] * _DIV
    E = np.stack([np.sin(om), np.cos(om)], -1).reshape(grid_n, HIDDEN)
    w = np.where((xg >= tlo) & (xg <= thi), 1.0, 0.05)
    C, *_ = np.linalg.lstsq(B * w[:, None], E * w[:, None], rcond=None)
    return C  # (m, 256)


_C_D = _fit_fourier(LO_D, HI_D, M_D, 0.1, 37.0)
_C_A = _fit_fourier(LO_A, HI_A, M_A, -0.05, 12.05)

_NC_CACHE = {}

NBLK = 8
BCH = 8           # chunks per block


def _build_nc():
    nc = bacc.Bacc("TRN2", target_bir_lowering=False, debug=False,
                   num_devices=NCORES)
    pts = nc.declare_dram_parameter("pts", [128, 12], F32, isOutput=False)
    nab = nc.declare_dram_parameter("nab", [128, 6, K], F32, isOutput=False)
    wlhs = nc.declare_dram_parameter("wlhs", [MB, 512], F32R, isOutput=False)
    jph = nc.declare_dram_parameter("jph", [MB, 2], F32, isOutput=False)
    biasd = nc.declare_dram_parameter("biasd", [128, 2], F32, isOutput=False)
    outT = nc.declare_dram_parameter("outT", [2, 128, 512], F32, isOutput=True)

    with tile.TileContext(nc) as tc:
        with (
            tc.tile_pool(name="singles", bufs=1) as sg,
            tc.tile_pool(name="geom", bufs=1) as gm,
            tc.tile_pool(name="dram", bufs=1, space="DRAM") as dr,
            tc.tile_pool(name="psum", bufs=1, space="PSUM") as pp,
            tc.tile_pool(name="thb", bufs=2) as tbp,
            tc.tile_pool(name="tblk", bufs=2) as tck,
            tc.tile_pool(name="iblk", bufs=2) as ick,
            tc.tile_pool(name="bblk", bufs=3) as bck,
        ):
            pts_sb = sg.tile([128, 12], F32, name="pts_sb")
            nab_sb = sg.tile([128, 6, K], F32, name="nab_sb")
            wlhs_sb = sg.tile([MB, 512], F32R, name="wlhs_sb")
            jph_sb = sg.tile([MB, 2], F32, name="jph_sb")
            bias_sb = sg.tile([128, 2], F32, name="bias_sb")
            mx_all = sg.tile([128, 4, 512], F32, name="mx_all")
            thrd = dr.tile([2, 8, 4096], F32, name="thrd")

            nc.gpsimd.dma_start(pts_sb[:], pts[:])
            nc.gpsimd.dma_start(nab_sb[:], nab[:])
            nc.gpsimd.dma_start(wlhs_sb[:], wlhs[:])
            nc.gpsimd.dma_start(jph_sb[:], jph[:])
            nc.gpsimd.dma_start(bias_sb[:], biasd[:])

            # ---------- geometry (per point-group g, 64 k-columns) ----------
            W = 4 * K
            u6 = gm.tile([128, 6, W], F32, name="u6")
            sq = gm.tile([128, 3, W], F32, name="sq")
            dd = gm.tile([128, W], F32, name="dd")      # dist^2 then scratch
            atd = gm.tile([128, W], F32, name="atd")    # dist
            ata = gm.tile([128, W], F32, name="ata")    # angle
            cx = gm.tile([128, 3, W], F32, name="cx")
            cc_ = gm.tile([128, W], F32, name="cc_")
            dt_ = gm.tile([128, W], F32, name="dt_")
            rc_ = gm.tile([128, W], F32, name="rc_")

            def geo(g):
                gg = slice(g * K, (g + 1) * K)
                # u6[c] = nab[c] + pts[:, group-col]  (DVE)
                for c in range(6):
                    nc.vector.tensor_scalar_add(
                        u6[:, c, gg], nab_sb[:, c, :],
                        pts_sb[:, g * 3 + (c % 3):g * 3 + (c % 3) + 1])
                # d^2 = sum of squares (Pool squares, DVE adds)
                for c in range(3):
                    nc.gpsimd.tensor_tensor(out=sq[:, c, gg], in0=u6[:, c, gg],
                                            in1=u6[:, c, gg], op=OP.mult)
                nc.vector.tensor_tensor(out=dd[:, gg], in0=sq[:, 0, gg],
                                        in1=sq[:, 1, gg], op=OP.add)
                nc.vector.tensor_tensor(out=dd[:, gg], in0=dd[:, gg],
                                        in1=sq[:, 2, gg], op=OP.add)
                # cross(u, v), its squares (Pool)
                for (i0, i1, i2, i3, o) in ((1, 5, 2, 4, 0), (2, 3, 0, 5, 1),
                                            (0, 4, 1, 3, 2)):
                    nc.gpsimd.tensor_tensor(out=sq[:, 0, gg] if o else cx[:, 0, gg],
                                            in0=u6[:, i0, gg], in1=u6[:, i1, gg],
                                            op=OP.mult)
                    nc.gpsimd.tensor_tensor(out=sq[:, 1, gg] if o else cx[:, 1, gg],
                                            in0=u6[:, i2, gg], in1=u6[:, i3, gg],
                                            op=OP.mult)
                    if o == 0:
                        nc.gpsimd.tensor_tensor(out=cx[:, 0, gg],
                                                in0=cx[:, 0, gg],
                                                in1=cx[:, 1, gg],
                                                op=OP.subtract)
                        nc.gpsimd.tensor_tensor(out=cx[:, 0, gg],
                                                in0=cx[:, 0, gg],
                                                in1=cx[:, 0, gg], op=OP.mult)
                        nc.vector.tensor_scalar(out=cc_[:, gg], in0=cx[:, 0, gg],
                                                scalar1=0.0, scalar2=None,
                                                op0=OP.add)
                    else:
                        nc.gpsimd.tensor_tensor(out=sq[:, 0, gg],
                                                in0=sq[:, 0, gg],
                                                in1=sq[:, 1, gg],
                                                op=OP.subtract)
                        nc.gpsimd.tensor_tensor(out=sq[:, 0, gg],
                                                in0=sq[:, 0, gg],
                                                in1=sq[:, 0, gg], op=OP.mult)
                        nc.vector.tensor_tensor(out=cc_[:, gg], in0=cc_[:, gg],
                                                in1=sq[:, 0, gg], op=OP.add)
                # dot(u, v) (Pool mult, DVE add)
                for c in range(3):
                    nc.gpsimd.tensor_tensor(out=cx[:, c, gg], in0=u6[:, c, gg],
                                            in1=u6[:, c + 3, gg], op=OP.mult)
                nc.vector.tensor_tensor(out=dt_[:, gg], in0=cx[:, 0, gg],
                                        in1=cx[:, 1, gg], op=OP.add)
                nc.vector.tensor_tensor(out=dt_[:, gg], in0=dt_[:, gg],
                                        in1=cx[:, 2, gg], op=OP.add)
                # sqrts back-to-back (one ACT table context)
                nc.scalar.activation(atd[:, gg], dd[:, gg], AF.Sqrt)
                nc.scalar.activation(cc_[:, gg], cc_[:, gg], AF.Sqrt)
                # angle = arctan(|c| / dot) + pi*(dot < 0)
                nc.vector.reciprocal(rc_[:, gg], dt_[:, gg])
                nc.vector.tensor_tensor(out=rc_[:, gg], in0=cc_[:, gg],
                                        in1=rc_[:, gg], op=OP.mult)
                nc.scalar.activation(rc_[:, gg], rc_[:, gg], AF.Arctan)
                nc.vector.tensor_scalar(out=dt_[:, gg], in0=dt_[:, gg],
                                        scalar1=0.0, scalar2=None, op0=OP.is_lt)
                nc.vector.scalar_tensor_tensor(out=ata[:, gg], in0=dt_[:, gg],
                                               scalar=float(np.pi),
                                               in1=rc_[:, gg],
                                               op0=OP.mult, op1=OP.add)
                # relayout to DRAM theta rows
                for h in range(2):
                    rr = 2 * g + h
                    src_d = atd[64 * h:64 * h + 64, gg]
                    src_a = ata[64 * h:64 * h + 64, gg]
                    dst_d = thrd[0:1, rr, :].rearrange("a (p k) -> a p k", k=K)
                    dst_a = thrd[1:2, rr, :].rearrange("a (p k) -> a p k", k=K)
                    nc.sync.dma_start(out=dst_d, in_=src_d)
                    nc.sync.dma_start(out=dst_a, in_=src_a)

            # ---------- block pipeline ------------------------------------
            thb = {}
            tt = {}
            it = {}
            bt = {}

            PS = pp.tile([128, 8, 512], F32, name="PS")

            def s_bcast(r):
                # dist rows -> partitions 0:96, angle rows -> 96:128
                tile_ = tbp.tile([MB, 8, 512], F32, name="thb")
                d_src = thrd[0:1, r, :]
                a_src = thrd[1:2, r, :]
                for i in range(8):
                    src = d_src if i < 4 else a_src
                    ap = bass.AP(tensor=src.tensor, offset=src.offset,
                                 ap=[[0, 16]] + list(src.ap)[1:])
                    p0 = i * 16
                    nc.sync.dma_start(
                        out=tile_[p0:p0 + 16, :, :].rearrange("p a b -> p (a b)"),
                        in_=ap)
                thb[r] = tile_

            def s_basis(r, h0=0, nh=8):
                # t = a_j*theta + b_j; i = round(t); t -= i; s = sin(2*pi*t)
                if h0 == 0:
                    tt[r] = tck.tile([MB, 8, 512], F32, name="tt")
                    bt[r] = bck.tile([MB, 8, 512], F32R, name="bt")
                    it[r] = ick.tile([MB, 8, 512], I32, name="it")
                t_, i_, b_ = tt[r], it[r], bt[r]
                src = thb[r][:, h0:h0 + nh, :].rearrange("p a b -> p (a b)")
                tv = t_[:, h0:h0 + nh, :].rearrange("p a b -> p (a b)")
                iv = i_[:, h0:h0 + nh, :].rearrange("p a b -> p (a b)")
                bv = b_[:, h0:h0 + nh, :].rearrange("p a b -> p (a b)")
                nc.scalar.activation(tv, src, AF.Identity,
                                     bias=jph_sb[:, 1:2], scale=jph_sb[:, 0:1])
                nc.scalar.copy(iv, tv)
                nc.gpsimd.tensor_tensor(out=tv, in0=tv, in1=iv, op=OP.subtract)
                nc.scalar.activation(bv, tv, AF.Sin, scale=TWO_PI)

            WSETS = [
                (wlhs_sb[0:M_D, 0:128], 0, M_D, 0),      # d0
                (wlhs_sb[0:M_D, 128:256], 0, M_D, 1),    # d1
                (wlhs_sb[M_D:MB, 256:384], M_D, MB, 2),  # a0
                (wlhs_sb[M_D:MB, 384:512], M_D, MB, 3),  # a1
            ]

            def s_mm(b, s, cc):
                wv, p0, p1, _ = WSETS[s]
                nc.tensor.matmul(PS[:, cc, :], wv, bt[b][p0:p1, cc, :],
                                 start=True, stop=True)

            def s_reduce(b, s, half):
                _, _, _, row = WSETS[s]
                c0 = b * BCH + half * 4
                nc.vector.tensor_reduce(
                    mx_all[:, row, c0 * 8:(c0 + 4) * 8],
                    PS[:, half * 4:half * 4 + 4, :].rearrange(
                        "p c (n k) -> p (c n) k", k=K),
                    axis=mybir.AxisListType.X, op=OP.max)

            # ---- emission ----
            geo(0)
            s_bcast(0)
            s_basis(0, 0, 4)
            s_basis(0, 4, 4)
            s_bcast(1)
            s_basis(1)
            for b in range(NBLK):
                nb = b + 2
                if nb < NBLK:
                    if nb % 2 == 0:
                        geo(nb // 2)
                    s_bcast(nb)
                    s_basis(nb)
                for s in range(4):
                    for cc in range(BCH):
                        s_mm(b, s, cc)
                    s_reduce(b, s, 0)
                    s_reduce(b, s, 1)
                thb.pop(b, None)
                tt.pop(b, None)
                it.pop(b, None)
                bt.pop(b, None)

            # ---------- finale ---------------------------------------------
            o0 = gm.tile([128, 512], F32, name="o0")
            o1 = gm.tile([128, 512], F32, name="o1")
            nc.vector.scalar_tensor_tensor(out=o0[:], in0=mx_all[:, 0, :],
                                           scalar=bias_sb[:, 0:1],
                                           in1=mx_all[:, 2, :],
                                           op0=OP.add, op1=OP.add)
            nc.vector.scalar_tensor_tensor(out=o1[:], in0=mx_all[:, 1, :],
                                           scalar=bias_sb[:, 1:2],
                                           in1=mx_all[:, 3, :],
                                           op0=OP.add, op1=OP.add)
            nc.sync.dma_start(out=outT[0], in_=o0[:])
            nc.sync.dma_start(out=outT[1], in_=o1[:])

    nc.compile()
    return nc


def _host_inputs(points, anchor_points, cor_score, Wa, ba, Wd, bd):
    p = np.ascontiguousarray(points[0], dtype=np.float32)       # (4096, 3)
    a = np.ascontiguousarray(anchor_points[0], dtype=np.float32)  # (64, 3)

    nab = np.empty((128, 6, K), np.float32)
    nab[:, 0:3, :] = -a.T[None, :, :]
    nab[:, 3:6, :] = -np.roll(a, -1, axis=0).T[None, :, :]

    G_d = (_C_D @ np.asarray(Wd).T).astype(np.float32)   # (M_D, 256)
    G_a = (_C_A @ np.asarray(Wa).T).astype(np.float32)   # (M_A, 256)
    wlhs = np.zeros((MB, 512), np.float32)
    wlhs[0:M_D, 0:128] = G_d[:, 0:128]
    wlhs[0:M_D, 128:256] = G_d[:, 128:256]
    wlhs[M_D:MB, 256:384] = G_a[:, 0:128]
    wlhs[M_D:MB, 384:512] = G_a[:, 128:256]

    # basis_j(x) = cos(j*pi*u) = sin(2*pi*t), t = (j/2)*u + 1/4,
    # u = (x - lo)/(hi - lo); x_d = 5*dist, x_a = FACTOR_A*angle.
    jph = np.zeros((MB, 2), np.float32)
    jd = np.arange(M_D); ja = np.arange(M_A)
    jph[0:M_D, 0] = 5.0 * jd / (2 * (HI_D - LO_D))
    jph[0:M_D, 1] = -jd * LO_D / (2 * (HI_D - LO_D)) + 0.25
    jph[M_D:MB, 0] = FACTOR_A * ja / (2 * (HI_A - LO_A))
    jph[M_D:MB, 1] = -ja * LO_A / (2 * (HI_A - LO_A)) + 0.25

    bsum = (np.asarray(bd) + np.asarray(ba)).astype(np.float32)
    biasd = np.stack([bsum[0:128], bsum[128:256]], axis=1).copy()  # (128, 2)

    in_maps = []
    for core in range(NCORES):
        pc = p[core * NC_PTS:(core + 1) * NC_PTS]   # (512, 3)
        ptsv = pc.reshape(4, 128, 3).transpose(1, 0, 2).reshape(128, 12)
        in_maps.append({
            "pts": np.ascontiguousarray(ptsv),
            "nab": nab,
            "wlhs": wlhs,
            "jph": jph,
            "biasd": biasd,
        })
    return in_maps


def kernel(points, anchor_points, cor_score, Wa, ba, Wd, bd, _timing=None,
           _trace=False, _trace_out=None):
    if "nc" not in _NC_CACHE:
        _NC_CACHE["nc"] = _build_nc()
    nc = _NC_CACHE["nc"]
    in_maps = _host_inputs(points, anchor_points, cor_score, Wa, ba, Wd, bd)
    res = run_bass_kernel_spmd(nc, in_maps, core_ids=list(range(NCORES)),
                               trace=_trace)
    if _trace_out is not None:
        _trace_out.append(res)
    if _timing is not None:
        _timing.append(res.exec_time_ns)
    out = np.empty((N, HIDDEN), np.float32)
    for core in range(NCORES):
        ot = res.results[core]["outT"]          # (2, 128, 512)
        blk = out[core * NC_PTS:(core + 1) * NC_PTS]
        blk[:, 0:128] = ot[0].T
        blk[:, 128:256] = ot[1].T
    return out.reshape(1, N, HIDDEN)
